# revision 17
# baseline (speedup 1.0000x reference)
"""Dynamic Directional Attention on 8 trn2 NeuronCores (Bass/Tile).

Problem: B=4, L=S=2048, H=8, E=64, f32.
  qt = tanh(q * 1/(std_H(q)+eps) * dw) * dyn     (std over the HEAD dim, ddof=1:
                                                  reference does std(axis=-2) on
                                                  [B,L,H,E], i.e. over H=8)
  kt likewise; scores[b,h,l,s] = qt . kt          (contract E)
  tau[l] = sqrt(var_s(scores[l,:], ddof=1) + eps)
  A = softmax(scale * scores / tau);  out = A @ v  [B,L,H,E]

Sharding: the head-std couples all 8 heads, so shard 8 cores = 4 batches x 2
L-halves. Each core gets q[b, half] = [1024, 512] and the full k/v[b] =
[2048, 512] (replicated across the half-pair), all heads contiguous in the
free dim - clean 2KB-row DMAs, no collectives.

Per-core kernel:
  - transform in natural layout: per l-row, 8-head strided tensor_reduce for
    sum/sumsq -> var -> rstd[l,e]; q*rstd (head-broadcast AP) -> tanh (ACT,
    scale=dw) -> bf16
  - PE-transpose transformed q,k into [e,l] per head-pair (2 heads per 128-row
    transpose)
  - pass 1 per head: S1[l,s] = tq @ tk^T (bf16, K=64) -> PSUM; bn_stats row
    var -> tau -> m[l] = scale*dyn^2/tau[l]   (scores = dyn^2 * S1)
  - fold m into q: qts[e,l] = tq[e,l]*m[l] (m transposed to a row via PE, DRAM
    bounce, broadcast-DMA), then st[s,l] = tk_chunk^T @ qts -> PSUM -> Exp on
    ACT -> A^T bf16. No max-subtraction needed: scaled scores have std 0.125.
  - A@V with V augmented by a ones column: out^T[d,l] accumulates over
    s-chunks; row 64 = softmax denominator. PE-transpose back to [l, 65],
    reciprocal + per-partition scale on DVE -> normalized output.
"""

import os
import sys

for _p in ("/opt/trn_rl_repo", "/root/.axon_site/_ro/trn_rl_repo"):
    if os.path.isdir(_p) and _p not in sys.path:
        sys.path.append(_p)

import numpy as np

import concourse.bass as bass
import concourse.mybir as mybir
import concourse.tile as tile
from concourse import bacc
from concourse.bass_utils import run_bass_kernel_spmd
from concourse.masks import make_identity

F32 = mybir.dt.float32
BF16 = mybir.dt.bfloat16
AF = mybir.ActivationFunctionType

B, L, S, H, E = 4, 2048, 2048, 8, 64
LC = L // 2          # 1024 l-rows per core
D = H * E            # 512 free-dim columns per core (all 8 heads)
P = 128
NLT = LC // P        # 8 l-chunks
NST = S // P         # 16 s-chunks
NLB = 2              # l-blocks of 512 for the st/AV phase
LB = 512
NHP = H // 2         # 4 head-pairs
EPS = 1e-6
SCALE = 1.0 / np.sqrt(E)
UNB_H = float(H) / float(H - 1)  # ddof=1 over heads
UNB_S = float(S) / float(S - 1)  # ddof=1 over score rows

_last_exec_time_ns = None


def _ensure_axon_hooks():
    """Provide antenv.axon_hooks (NTFF profiling hook) if the image lacks it.

    Mirrors trn_agent_boot.trn_boot's ctypes shim against libaxon_pjrt.so.
    Only used when BASS_TRACE is set; harmless otherwise.
    """
    try:
        import antenv.axon_hooks  # noqa: F401

        return
    except ImportError:
        pass
    import contextlib
    import ctypes
    import types

    try:
        import antenv
    except ImportError:
        return

    holder = {"h": None}
    mod = types.ModuleType("antenv.axon_hooks")
    mod.set_axon_ntff_profile_hook = lambda h: holder.__setitem__("h", h)
    mod.get_axon_ntff_profile_hook = lambda: holder["h"]
    sys.modules["antenv.axon_hooks"] = mod
    antenv.axon_hooks = mod

    so_path = "/opt/axon/libaxon_pjrt.so"
    if not os.path.exists(so_path):
        return
    try:
        lib = ctypes.CDLL(so_path)
    except OSError:
        return
    if not hasattr(lib, "axon_start_nrt_profile"):
        return
    lib.axon_start_nrt_profile.argtypes = [
        ctypes.POINTER(ctypes.c_int64),
        ctypes.c_size_t,
    ]
    lib.axon_start_nrt_profile.restype = ctypes.c_int64
    lib.axon_stop_nrt_profile.argtypes = [ctypes.c_char_p]
    lib.axon_stop_nrt_profile.restype = ctypes.c_int64

    @contextlib.contextmanager
    def _hook(output_dir, device_ids):
        import jax

        jax.devices()
        if device_ids:
            ids = (ctypes.c_int64 * len(device_ids))(*device_ids)
            rc = lib.axon_start_nrt_profile(ids, len(device_ids))
        else:
            rc = lib.axon_start_nrt_profile(None, 0)
        if rc != 0:
            raise RuntimeError(f"axon_start_nrt_profile rc={rc}")
        try:
            yield
        finally:
            n = lib.axon_stop_nrt_profile(str(output_dir).encode())
            print(f"profile: {n} file(s) written to {output_dir}", file=sys.stderr)

    holder["h"] = _hook


def _head_bcast(ap_2d, nh=H, ne=E):
    """View a [p, ne] AP as [p, nh, ne] with the head dim broadcast (step 0)."""
    return bass.AP(
        tensor=ap_2d.tensor,
        offset=ap_2d.offset,
        ap=[list(ap_2d.ap[0]), [0, nh], list(ap_2d.ap[1])],
    )


def build_nc():
    nc = bacc.Bacc("TRN2", target_bir_lowering=False, debug=False)
    q_d = nc.dram_tensor("q", [LC, D], F32, kind="ExternalInput")
    k_d = nc.dram_tensor("k", [S, D], F32, kind="ExternalInput")
    v_d = nc.dram_tensor("v", [S, D], F32, kind="ExternalInput")
    dw_d = nc.dram_tensor("dw", [1, 1], F32, kind="ExternalInput")
    dp_d = nc.dram_tensor("dp", [1, 1], F32, kind="ExternalInput")
    o_d = nc.dram_tensor("o", [LC, D], F32, kind="ExternalOutput")

    q_r = q_d.rearrange("(n p) d -> p n d", p=P)
    k_r = k_d.rearrange("(n p) d -> p n d", p=P)
    v_r = v_d.rearrange("(n p) d -> p n d", p=P)
    o_r = o_d.rearrange("(n p) d -> p n d", p=P)

    from contextlib import ExitStack

    with tile.TileContext(nc) as tc, ExitStack() as ctx:
        ek = ctx.enter_context
        sing = ek(tc.tile_pool(name="sing", bufs=1))
        pqn = ek(tc.tile_pool(name="qn", bufs=4))        # [128,512]f32 stream
        pkn = ek(tc.tile_pool(name="kn", bufs=4))
        pvn = ek(tc.tile_pool(name="vn", bufs=4))
        ptn = ek(tc.tile_pool(name="tn", bufs=4))        # transformed nat bf16
        pqt = ek(tc.tile_pool(name="qt", bufs=NHP))      # tqT/tkT per pair
        pqts = ek(tc.tile_pool(name="qts", bufs=2))
        pmb = ek(tc.tile_pool(name="mb", bufs=2))
        pat = ek(tc.tile_pool(name="at", bufs=2))        # A^T block bf16
        pva = ek(tc.tile_pool(name="va", bufs=H))        # Vaug per head
        pout = ek(tc.tile_pool(name="outp", bufs=1))
        pot = ek(tc.tile_pool(name="ot", bufs=3))
        pmr = ek(tc.tile_pool(name="mrow", bufs=2))
        psc = ek(tc.tile_pool(name="small", bufs=4))
        pst = ek(tc.tile_pool(name="stat", bufs=2))      # [128,512] f32 scratch
        pdr = ek(tc.tile_pool(name="dr", bufs=2, space="DRAM"))
        pps = ek(tc.tile_pool(name="ps", bufs=2, space="PSUM"))
        ppo = ek(tc.tile_pool(name="po", bufs=2, space="PSUM"))
        ppt = ek(tc.tile_pool(name="ptr", bufs=2, space="PSUM"))
        pptb = ek(tc.tile_pool(name="ptrb", bufs=2, space="PSUM"))

        # --- constants ---
        ident = sing.tile([P, P], BF16)
        make_identity(nc, ident)
        identf = sing.tile([P, P], F32)
        make_identity(nc, identf)
        zero_t = sing.tile([P, 1], F32)
        nc.vector.memset(zero_t, 0.0)
        eps_t = sing.tile([P, 1], F32)
        nc.vector.memset(eps_t, EPS)
        dw_t = sing.tile([P, 1], F32)
        nc.sync.dma_start(out=dw_t, in_=dw_d[:, :].to_broadcast([P, 1]))
        dp_t = sing.tile([P, 1], F32)
        nc.sync.dma_start(out=dp_t, in_=dp_d[:, :].to_broadcast([P, 1]))
        dp2 = sing.tile([P, 1], F32)
        nc.vector.tensor_mul(dp2, dp_t, dp_t)
        c2 = sing.tile([P, 1], F32)  # scale * dyn^2
        nc.vector.tensor_scalar_mul(c2, dp2, float(SCALE))
        dp4 = sing.tile([P, 1], F32)
        nc.vector.tensor_mul(dp4, dp2, dp2)
        c4 = sing.tile([P, 1], F32)  # dyn^4 * unbiased(S) factor
        nc.vector.tensor_scalar_mul(c4, dp4, UNB_S)

        def transform_chunk(nat, tnat):
            """tnat = tanh(nat * rstd(headwise std) * dw), [128, D] -> bf16."""
            he = nat.rearrange("p (h e) -> p e h", h=H)  # heads innermost
            ssum = psc.tile([P, E], F32, tag="ssum")
            nc.vector.tensor_reduce(ssum, he, axis=mybir.AxisListType.X,
                                    op=mybir.AluOpType.add)
            sq = pst.tile([P, D], F32, tag="sq")
            nc.scalar.activation(sq, nat, AF.Square, bias=zero_t, scale=1.0)
            ssq = psc.tile([P, E], F32, tag="ssq")
            nc.vector.tensor_reduce(ssq, sq.rearrange("p (h e) -> p e h", h=H),
                                    axis=mybir.AxisListType.X,
                                    op=mybir.AluOpType.add)
            mean = psc.tile([P, E], F32, tag="mean")
            nc.vector.tensor_scalar_mul(mean, ssum, 1.0 / H)
            m2 = psc.tile([P, E], F32, tag="m2")
            nc.vector.tensor_mul(m2, mean, mean)
            var = psc.tile([P, E], F32, tag="var")
            nc.vector.tensor_scalar_mul(var, ssq, 1.0 / H)
            nc.vector.tensor_sub(var, var, m2)
            stdv = psc.tile([P, E], F32, tag="stdv")
            nc.scalar.activation(stdv, var, AF.Sqrt, bias=zero_t, scale=UNB_H)
            nc.vector.tensor_scalar_add(stdv, stdv, EPS)
            rstd = psc.tile([P, E], F32, tag="rstd")
            nc.vector.reciprocal(rstd, stdv)
            tmp = pst.tile([P, D], F32, tag="tmp")
            nc.vector.tensor_mul(tmp, nat, _head_bcast(rstd[:, :]))
            nc.scalar.activation(tnat, tmp, AF.Tanh, bias=zero_t, scale=dw_t)

        # --- load + transform + transpose q,k into packed [2*E, l] per pair ---
        tqT = []
        tkT = []
        for _hp in range(NHP):
            qT_t = pqt.tile([P, LC], BF16, tag="tqT")
            tqT.append(qT_t)
            kT_t = pqt.tile([P, S], BF16, tag="tkT")
            tkT.append(kT_t)
        for i in range(NLT):
            qn = pqn.tile([P, D], F32, tag="qn")
            nc.sync.dma_start(out=qn, in_=q_r[:, i, :])
            tn = ptn.tile([P, D], BF16, tag="tqn")
            transform_chunk(qn, tn)
            for hp in range(NHP):
                pt = pptb.tile([P, P], BF16, tag="tpb")
                nc.tensor.transpose(pt, tn[:, hp * P : (hp + 1) * P], ident)
                nc.vector.tensor_copy(tqT[hp][:, i * P : (i + 1) * P], pt)
        for i in range(NST):
            kn = pkn.tile([P, D], F32, tag="kn")
            nc.sync.dma_start(out=kn, in_=k_r[:, i, :])
            tn = ptn.tile([P, D], BF16, tag="tkn")
            transform_chunk(kn, tn)
            for hp in range(NHP):
                pt = pptb.tile([P, P], BF16, tag="tpb")
                nc.tensor.transpose(pt, tn[:, hp * P : (hp + 1) * P], ident)
                nc.vector.tensor_copy(tkT[hp][:, i * P : (i + 1) * P], pt)

        # --- V + ones column, bf16, per head ---
        vaug = []
        for h in range(H):
            va = pva.tile([P, NST, E + 1], BF16, tag="va")
            nc.vector.memset(va[:, :, E : E + 1], 1.0)
            vaug.append(va)
        for kk in range(NST):
            vn = pvn.tile([P, D], F32, tag="vn")
            nc.sync.dma_start(out=vn, in_=v_r[:, kk, :])
            for h in range(H):
                nc.vector.tensor_copy(vaug[h][:, kk, 0:E],
                                      vn[:, h * E : (h + 1) * E])

        out_all = pout.tile([P, NLT, D], F32, tag="outp")

        for h in range(H):
            hp, local = h // 2, h % 2
            off = local * E
            tq = tqT[hp]
            tk = tkT[hp]

            # --- pass 1: S1[l,s] row stats -> m[l] ---
            mmat = psc.tile([P, NLT], F32, tag="mmat")
            for i in range(NLT):
                st4 = psc.tile([P, 4, 6], F32, tag="s1st")
                for j in range(4):
                    ps = pps.tile([P, 512], F32, tag="ps")
                    nc.tensor.matmul(
                        ps,
                        tq[off : off + E, i * P : (i + 1) * P],
                        tk[off : off + E, j * 512 : (j + 1) * 512],
                        start=True, stop=True,
                    )
                    nc.vector.bn_stats(st4[:, j, :], ps)
                mv1 = psc.tile([P, 2], F32, tag="mv1")
                nc.vector.bn_aggr(mv1, st4)
                tau = psc.tile([P, 1], F32, tag="tau")
                nc.scalar.activation(tau, mv1[:, 1:2], AF.Sqrt,
                                     bias=eps_t, scale=c4)
                rtau = psc.tile([P, 1], F32, tag="rtau")
                nc.vector.reciprocal(rtau, tau)
                nc.vector.tensor_mul(mmat[:, i : i + 1], rtau, c2)

            # --- m[l] -> broadcast row, fold into q ---
            ptm = ppt.tile([P, P], F32, tag="tp")
            nc.tensor.transpose(ptm[0:NLT, :], mmat, identf)
            mT = pmr.tile([NLT, P], BF16, tag="mT")
            nc.vector.tensor_copy(mT, ptm[0:NLT, :])
            mdr = pdr.tile([1, LC], BF16, tag="mdr")
            nc.sync.dma_start(out=mdr.rearrange("a (b c) -> a b c", b=NLT), in_=mT)
            mb = pmb.tile([P, LC], BF16, tag="mb")
            nc.sync.dma_start(out=mb[off : off + E, :],
                              in_=mdr.to_broadcast([E, LC]))
            qts = pqts.tile([P, LC], BF16, tag="qts")
            nc.vector.tensor_mul(qts[off : off + E, :], tq[off : off + E, :],
                                 mb[off : off + E, :])

            # --- st[s,l] -> exp -> A^T; A^T @ Vaug -> out^T; normalize ---
            for lb in range(NLB):
                at = pat.tile([P, NST, LB], BF16, tag="at")
                for kk in range(NST):
                    ps2 = pps.tile([P, 512], F32, tag="ps")
                    nc.tensor.matmul(
                        ps2,
                        tk[off : off + E, kk * P : (kk + 1) * P],
                        qts[off : off + E, lb * LB : (lb + 1) * LB],
                        start=True, stop=True,
                    )
                    nc.scalar.activation(at[:, kk, :], ps2, AF.Exp,
                                         bias=zero_t, scale=1.0)
                po = ppo.tile([E + 1, LB], F32, tag="po")
                for kk in range(NST):
                    nc.tensor.matmul(po, vaug[h][:, kk, :], at[:, kk, :],
                                     start=(kk == 0), stop=(kk == NST - 1))
                ot = pot.tile([E + 1, LB], F32, tag="ot")
                nc.vector.tensor_copy(ot, po)
                for t in range(LB // P):
                    ptt = ppt.tile([P, P], F32, tag="tp")
                    nc.tensor.transpose(ptt[:, 0 : E + 1],
                                        ot[:, t * P : (t + 1) * P],
                                        identf[0 : E + 1, 0 : E + 1])
                    rec = psc.tile([P, 1], F32, tag="rec")
                    nc.vector.reciprocal(rec, ptt[:, E : E + 1])
                    li = lb * (LB // P) + t
                    nc.vector.tensor_scalar_mul(
                        out_all[:, li, h * E : (h + 1) * E], ptt[:, 0:E], rec
                    )

        for i in range(NLT):
            nc.sync.dma_start(out=o_r[:, i : i + 1, :],
                              in_=out_all[:, i : i + 1, :])

    return nc


_nc_cache = None


def kernel(queries, keys, values, attn_mask=None, directional_weights=None,
           dynamic_param=None, **_unused):
    global _nc_cache, _last_exec_time_ns
    q = np.asarray(queries, dtype=np.float32)
    k = np.asarray(keys, dtype=np.float32)
    v = np.asarray(values, dtype=np.float32)
    dw = np.asarray(directional_weights, dtype=np.float32).reshape(1, 1)
    dp = np.asarray(dynamic_param, dtype=np.float32).reshape(1, 1)

    if _nc_cache is None:
        nc = build_nc()
        nc.finalize()
        _nc_cache = nc
    nc = _nc_cache

    in_maps = []
    for c in range(8):
        b, lh = c // 2, c % 2
        in_maps.append({
            "q": np.ascontiguousarray(q[b, lh * LC : (lh + 1) * LC]).reshape(LC, D),
            "k": np.ascontiguousarray(k[b]).reshape(S, D),
            "v": np.ascontiguousarray(v[b]).reshape(S, D),
            "dw": dw, "dp": dp,
        })

    tracing = bool(os.environ.get("BASS_TRACE"))
    if tracing:
        _ensure_axon_hooks()
        import concourse.bass_utils as _bu

        _orig_upload = _bu.upload_artifacts
        _bu.upload_artifacts = lambda d: d  # no bucket access in this sandbox
        try:
            res = run_bass_kernel_spmd(nc, in_maps, core_ids=list(range(8)))
        except Exception as e:  # fall back to an untraced run
            print(f"traced run failed ({e!r}); retrying untraced", file=sys.stderr)
            os.environ["BASS_NEVER_TRACE"] = "1"
            try:
                res = run_bass_kernel_spmd(nc, in_maps, core_ids=list(range(8)))
            finally:
                os.environ.pop("BASS_NEVER_TRACE", None)
        finally:
            _bu.upload_artifacts = _orig_upload
    else:
        res = run_bass_kernel_spmd(nc, in_maps, core_ids=list(range(8)))
    _last_exec_time_ns = res.exec_time_ns

    out = np.empty((B, L, H, E), dtype=np.float32)
    for c in range(8):
        b, lh = c // 2, c % 2
        out[b, lh * LC : (lh + 1) * LC] = res.results[c]["o"].reshape(LC, H, E)
    return out
